# revision 2
# baseline (speedup 1.0000x reference)
"""Trainium2 Bass kernel for nn_SanctionImpactGNN (2-hop pruned).

The model output is sigmoid(heads(GRU(seq))) where seq[t] = h2[node0] of a
2-layer GCN on graph t.  h2[node0] only depends on the 2-hop in-neighborhood
of node 0 (~350 edges of 320000), plus the weighted in-degrees (for the
symmetric normalization) of the ~350 involved nodes.

Host (indexing/packing only -- no FP math):
  * L1 = in-neighbors of node 0 (plus node 0 itself); layer-1 edge slots =
    all edges into L1 nodes + one self-loop slot per L1 node.
  * Per slot: the source node's x row (bf16 column of xT), in-edge weight
    lists of the slot's src and dst stacked along the partition (K) dim so a
    single matmul against a block-ones matrix computes all degree sums, the
    slot's own edge weight, and a one-hot dst indicator column.
  Everything is packed in ONE uint8 blob per graph -> one DMA per core.

Device phase 1 (SPMD, one graph per core):
  degs  = three tiny PE matmuls (lists^T @ block-ones) -> PSUM [128, 3nch+3]
  dis   = reciprocal(sqrt(1+degs))          (one ACT + one DVE op)
  norm  = dis[src]*dis[dst]*ew  per slot    (two DVE ops)
  msg_c = x[src]@W1 per slot chunk (bf16 matmul), copied to SBUF early
  psh1  = b1 + sum_c (A_c * norm_c)^T @ msg_c   (PSUM accumulation; b1 via
          ones-row x b1-row matmul)
  h1    = relu(psh1)                        (DVE tensor_relu)
  india = relu(b2 + dis0 * (W2^T @ (h1^T @ (disL1*s0))))  -> [64,1] f32

Device phase 2 (single core): GRU + heads (bf16 weights/h), per step:
  ps_rz = Whh_rz^T@h + Wih_rz^T@x_t   (PSUM accumulation, bias rows folded)
  sigmoid(ps_rz); tanh(ps_n*r + gi_n); h' = z*(h-n) + n
as 4 activation instructions + 3 tiny matmuls.
"""

import numpy as np
import ml_dtypes

import concourse.bacc as bacc
import concourse.mybir as mybir
import concourse.tile as tile
from concourse import bass_utils

F32 = mybir.dt.float32
BF16 = mybir.dt.bfloat16
U8 = mybir.dt.uint8
AF = mybir.ActivationFunctionType
OP = mybir.AluOpType
AX = mybir.AxisListType

T, N, E, F, H = 8, 20000, 320000, 128, 64
P = 128
INDIA = 0

BF = np.dtype(ml_dtypes.bfloat16)


def _align4(x):
    return (x + 3) & ~3


class Plan:
    """Compile-time sizes shared by all graphs (SPMD)."""

    def __init__(self, nch, nl1, k):
        self.nch, self.nl1, self.k = nch, nl1, k
        self.s1p = nch * P
        # degree-matmul grouping: gpm segments of length k per matmul (<=128
        # partitions each).  Segments: nch src + nch dst + degL1 + deg0 + s0.
        self.gpm = max(1, P // k)
        segs = 2 * nch + 3
        self.nmm = (segs + self.gpm - 1) // self.gpm
        self.segs = segs
        o = 0
        self.o_xt = o; o += self.s1p * 2                 # bf16 [128, S1P]
        self.o_dl = o; o += self.nmm * P * 2             # bf16 [gpm*k, nmm*128]
        self.o_on = o; o += self.gpm * self.segs * 2     # bf16 ones blocks
        self.o_a = o; o = _align4(o + nch * nl1 * 2)     # bf16 [128, nch*NL1]
        self.o_ew = o; o += nch * 4                      # f32  [128, nch]
        self.o_w1 = o; o += H * 2                        # bf16 [128, 64]
        self.o_w2 = o; o += H * 2                        # bf16 [64, 64]
        self.o_b1 = o; o += H * 2                        # bf16 [1, 64] row
        self.o_or = o; o = _align4(o + 2)                # bf16 [1, 1] one
        self.o_b2 = o; o = _align4(o + 4)                # f32  [64, 1]
        self.pb = _align4(o)

    def key(self):
        return (self.nch, self.nl1, self.k)


def graph_prep(x_t, src, dst, ew):
    """Per-graph host extraction (pure indexing).  Returns raw structures."""
    order = np.argsort(dst, kind="stable")
    dst_s = dst[order]
    starts = np.searchsorted(dst_s, np.arange(N + 1))

    def in_edges(v):          # edge ids (original) into node v
        return order[starts[v]:starts[v + 1]]

    e0 = in_edges(INDIA)
    l1rest = np.unique(src[e0])
    l1rest = l1rest[l1rest != INDIA]
    L1 = np.concatenate([[INDIA], l1rest]).astype(np.int64)
    nl1 = len(L1)

    slot_src, slot_dst, slot_ew = [], [], []
    for j, v in enumerate(L1):
        eids = in_edges(v)
        slot_src.append(src[eids])
        slot_dst.append(np.full(len(eids) + 1, j, np.int64))
        slot_ew.append(ew[eids])
        # self-loop
        slot_src.append(np.array([v], np.int64))
        slot_ew.append(np.array([1.0], np.float32))
    slot_src = np.concatenate(slot_src).astype(np.int64)
    slot_dst = np.concatenate(slot_dst)
    slot_ew = np.concatenate(slot_ew).astype(np.float32)
    s1 = len(slot_src)

    involved = np.unique(np.concatenate([slot_src, L1]))
    indeg = starts[1:] - starts[:-1]
    kmax = int(indeg[involved].max()) if len(involved) else 1
    # s0 lists: edges L1[j] -> 0 (+ self-loop 1.0 for j==0)
    s0_lists = []
    src_e0 = src[e0]
    for j, v in enumerate(L1):
        vals = ew[e0[src_e0 == v]]
        if v == INDIA:
            vals = np.concatenate([vals, [np.float32(1.0)]])
        s0_lists.append(np.asarray(vals, np.float32))
    kmax = max(kmax, max(len(v) for v in s0_lists))
    return dict(L1=L1, nl1=nl1, slot_src=slot_src, slot_dst=slot_dst,
                slot_ew=slot_ew, s1=s1, kmax=kmax, s0_lists=s0_lists,
                in_edges=in_edges, ew=ew, x_t=x_t)


def make_plan(preps):
    s1 = max(p["s1"] for p in preps)
    nch = (s1 + P - 1) // P
    nl1 = max(p["nl1"] for p in preps)
    k = max(p["kmax"] for p in preps)
    return Plan(nch, nl1, k)


def pack_graph(plan, prep, W1, W2, b1, b2):
    nch, nl1, k = plan.nch, plan.nl1, plan.k
    gpm, segs, nmm = plan.gpm, plan.segs, plan.nmm
    s1p = plan.s1p
    blob = np.zeros((P, plan.pb), np.uint8)

    def put(off, arr, rows=P):
        a = np.ascontiguousarray(arr)
        b = a.view(np.uint8).reshape(rows, -1)
        blob[:rows, off:off + b.shape[1]] = b

    ns = prep["s1"]
    ssrc, sdst, sew = prep["slot_src"], prep["slot_dst"], prep["slot_ew"]
    in_edges, ew = prep["in_edges"], prep["ew"]
    L1 = prep["L1"]

    # xT: [128 features, S1P slots] bf16
    xt = np.zeros((P, s1p), BF)
    xt[:, :ns] = np.asarray(prep["x_t"], np.float32)[ssrc].T.astype(BF)
    put(plan.o_xt, xt)

    # degree list segments, each [k, 128] (cols = lanes):
    #   seg c       (c<nch):    in-ew list of src(slot c,p) at col p
    #   seg nch+c:              in-ew list of dst(slot c,p) at col p
    #   seg 2nch:               in-ew list of L1[p]  (cols < nl1)
    #   seg 2nch+1:             in-ew list of node 0 (all 64 cols)
    #   seg 2nch+2:             ew of edges L1[p] -> 0 (+1.0 at p==0)
    seglists = np.zeros((segs, k, P), BF)
    lane, chunk = np.arange(ns) % P, np.arange(ns) // P
    l1arr = L1[sdst]
    for s in range(ns):
        lst = ew[in_edges(ssrc[s])]
        seglists[chunk[s], :len(lst), lane[s]] = lst.astype(BF)
        lstd = ew[in_edges(l1arr[s])]
        seglists[nch + chunk[s], :len(lstd), lane[s]] = lstd.astype(BF)
    for j, v in enumerate(L1):
        lst = ew[in_edges(v)]
        seglists[2 * nch, :len(lst), j] = lst.astype(BF)
        s0l = prep["s0_lists"][j]
        seglists[2 * nch + 2, :len(s0l), j] = s0l.astype(BF)
    l0 = ew[in_edges(INDIA)].astype(BF)
    seglists[2 * nch + 1, :len(l0), :H] = l0[:, None]

    # stack gpm segments per matmul along partitions; ones blocks map each
    # K-rows group to its psum column.
    dlm = np.zeros((P, nmm * P), BF)
    onem = np.zeros((P, gpm * segs), BF)
    for si in range(segs):
        m, g = si // gpm, si % gpm
        dlm[g * k:(g + 1) * k, m * P:(m + 1) * P] = seglists[si]
        onem[g * k:(g + 1) * k, si] = np.float32(1.0)
    put(plan.o_dl, dlm)
    put(plan.o_on, onem)

    A = np.zeros((P, nch, nl1), BF)
    ewm = np.zeros((P, nch), np.float32)
    ewm[lane, chunk] = sew
    A[lane, chunk, sdst] = np.float32(1.0)
    put(plan.o_a, A.reshape(P, -1))
    put(plan.o_ew, ewm)

    put(plan.o_w1, np.asarray(W1, np.float32).astype(BF))            # [128,64]
    put(plan.o_w2, np.asarray(W2, np.float32).astype(BF), rows=H)    # [64,64]
    put(plan.o_b1, np.asarray(b1, np.float32).astype(BF).reshape(1, H), rows=1)
    put(plan.o_or, np.ones((1, 1), BF), rows=1)
    put(plan.o_b2, np.asarray(b2, np.float32).reshape(H, 1), rows=H)
    return blob


def build_phase1(nc, plan, stage=99):
    nch, nl1, k = plan.nch, plan.nl1, plan.k
    gpm, segs, nmm = plan.gpm, plan.segs, plan.nmm
    pk_d = nc.dram_tensor("pk", [P, plan.pb], U8, kind="ExternalInput")
    india_d = nc.dram_tensor("india", [H, 1], F32, kind="ExternalOutput")
    dbg_d = None
    if stage < 99:
        dbg_d = nc.dram_tensor("dbg", [P, 256], F32, kind="ExternalOutput")

    with tile.TileContext(nc) as tc:
        with (
            tc.tile_pool(name="const", bufs=1) as const,
            tc.tile_pool(name="sm", bufs=1) as sm,
            tc.tile_pool(name="wm", bufs=3) as wm,
            tc.tile_pool(name="psm", bufs=3, space="PSUM") as psmp,
            tc.tile_pool(name="ps1", bufs=1, space="PSUM") as ps1,
        ):
            # warm the Sqrt/Relu/Copy act table while the blob DMA flies
            warm = const.tile([1, 1], F32, tag="warm")
            nc.vector.memset(warm[:], 1.0)
            nc.scalar.activation(warm[:], warm[:], AF.Sqrt)

            pk = const.tile([P, plan.pb], U8, tag="pk")
            nc.sync.dma_start(pk[:], pk_d[:])

            xv = pk[:, plan.o_xt:plan.o_xt + plan.s1p * 2].bitcast(BF16)
            dlv = pk[:, plan.o_dl:plan.o_dl + nmm * P * 2].bitcast(BF16)
            onv = pk[:, plan.o_on:plan.o_on + gpm * segs * 2].bitcast(BF16)
            av = pk[:, plan.o_a:plan.o_a + nch * nl1 * 2].bitcast(BF16)
            ewv = pk[:, plan.o_ew:plan.o_ew + nch * 4].bitcast(F32)
            w1v = pk[:, plan.o_w1:plan.o_w1 + H * 2].bitcast(BF16)
            w2v = pk[0:H, plan.o_w2:plan.o_w2 + H * 2].bitcast(BF16)
            b1v = pk[0:1, plan.o_b1:plan.o_b1 + H * 2].bitcast(BF16)
            onr = pk[0:1, plan.o_or:plan.o_or + 2].bitcast(BF16)
            b2v = pk[0:H, plan.o_b2:plan.o_b2 + 4].bitcast(F32)

            # --- degree sums via block-ones matmuls -> psd [128, segs]
            psd = ps1.tile([P, segs], F32, tag="psd")
            for m in range(nmm):
                lo, hi = m * gpm, min((m + 1) * gpm, segs)
                nsg = hi - lo
                nc.tensor.matmul(psd[:, lo:hi],
                                 dlv[0:gpm * k, m * P:(m + 1) * P],
                                 onv[0:gpm * k, lo:hi],
                                 start=True, stop=True)
            # dis = 1/sqrt(1+deg) for all but the raw-s0 column
            dsq = sm.tile([P, segs - 1], F32, tag="dsq")
            nc.scalar.activation(dsq[:], psd[:, 0:segs - 1], AF.Sqrt, bias=1.0)
            dis = sm.tile([P, segs - 1], F32, tag="dis")
            nc.vector.reciprocal(dis[:], dsq[:])
            # norm = dis_src * dis_dst * ew  [128, nch]
            norm = sm.tile([P, nch], F32, tag="norm")
            nc.vector.tensor_mul(norm[:], dis[:, 0:nch], dis[:, nch:2 * nch])
            nc.vector.tensor_mul(norm[:], norm[:], ewv)
            # cvec = disL1 * s0_raw (rows < nl1)
            cvec = sm.tile([H, 1], BF16, tag="cvec")
            nc.vector.tensor_mul(cvec[:], dis[0:H, 2 * nch:2 * nch + 1],
                                 psd[0:H, segs - 1:segs])

            if stage == 0:
                z = sm.tile([P, 256], F32, tag="dbgt")
                nc.vector.memset(z[:], 0.0)
                nc.vector.tensor_copy(z[:, 0:segs], psd[:])
                nc.vector.tensor_copy(z[:, 16:16 + nch], norm[:])
                nc.vector.tensor_copy(z[0:H, 24:25], cvec[:])
                nc.sync.dma_start(dbg_d[:], z[:])

            # --- per-chunk messages; copies off critical path
            psh1 = ps1.tile([nl1, H], F32, tag="psh1")
            # b1 enters the accumulation first (no data deps beyond blob)
            nc.tensor.matmul(psh1[:], onr.broadcast_to((1, nl1)), b1v,
                             start=True, stop=False)
            msgs = []
            for c in range(nch):
                psm = psmp.tile([P, H], F32, tag="psm")
                nc.tensor.matmul(psm[:], xv[:, c * P:(c + 1) * P], w1v,
                                 start=True, stop=True)
                msg = wm.tile([P, H], BF16, tag="msg")
                if c == 0:
                    nc.vector.tensor_copy(msg[:], psm[:])
                else:
                    nc.scalar.copy(msg[:], psm[:])
                msgs.append(msg)
            asc = sm.tile([P, nch * nl1], BF16, tag="asc")
            for c in range(nch):
                nc.vector.tensor_scalar_mul(
                    asc[:, c * nl1:(c + 1) * nl1],
                    av[:, c * nl1:(c + 1) * nl1], norm[:, c:c + 1])
                nc.tensor.matmul(psh1[:], asc[:, c * nl1:(c + 1) * nl1],
                                 msgs[c][:], start=False, stop=(c == nch - 1))

            h1 = sm.tile([nl1, H], BF16, tag="h1")
            nc.vector.tensor_relu(h1[:], psh1[:])

            if stage == 1:
                z = sm.tile([P, 256], F32, tag="dbgt")
                nc.vector.memset(z[:], 0.0)
                nc.vector.tensor_copy(z[0:nl1, 0:H], psh1[:])
                nc.sync.dma_start(dbg_d[:], z[:])

            ps_a = ps1.tile([H, 1], F32, tag="psa")
            nc.tensor.matmul(ps_a[:], h1[:], cvec[0:nl1, 0:1],
                             start=True, stop=True)
            agg1 = sm.tile([H, 1], BF16, tag="agg1")
            nc.vector.tensor_copy(agg1[:], ps_a[:])
            ps_h2 = ps1.tile([H, 1], F32, tag="psh2")
            nc.tensor.matmul(ps_h2[:], w2v, agg1[:], start=True, stop=True)
            india = sm.tile([H, 1], F32, tag="india")
            nc.scalar.activation(india[:], ps_h2[:], AF.Relu,
                                 scale=dis[0:H, 2 * nch + 1:2 * nch + 2],
                                 bias=b2v)
            nc.sync.dma_start(india_d[:], india[:])
    nc.compile()
    return nc


def build_phase2(nc):
    h, t_steps = H, T
    # packed bf16 blob rows 0..64: wihT_aug | whhT_aug | headWT_aug | xaug
    w2cols = 3 * h + 3 * h + 8 + t_steps
    pk_d = nc.dram_tensor("pk2", [P, w2cols * 2], U8, kind="ExternalInput")
    out_d = nc.dram_tensor("out", [8, 1], F32, kind="ExternalOutput")

    with tile.TileContext(nc) as tc:
        with (
            tc.tile_pool(name="const", bufs=1) as const,
            tc.tile_pool(name="sm", bufs=4) as sm,
            tc.tile_pool(name="ps", bufs=2, space="PSUM") as pspool,
            tc.tile_pool(name="ps1", bufs=1, space="PSUM") as ps1,
        ):
            warm = const.tile([1, 1], F32, tag="warm")
            nc.vector.memset(warm[:], 0.0)
            nc.scalar.activation(warm[:], warm[:], AF.Sigmoid)

            pk = const.tile([P, w2cols * 2], U8, tag="pk2")
            nc.sync.dma_start(pk[:], pk_d[:])
            fv = pk[0:h + 1, :].bitcast(BF16)
            wih = fv[:, 0:3 * h]
            whh = fv[:, 3 * h:6 * h]
            hw = fv[:, 6 * h:6 * h + 8]
            xaug = fv[:, 6 * h + 8:6 * h + 8 + t_steps]

            haug = const.tile([h + 1, 1], BF16, tag="haug")
            nc.vector.memset(haug[0:h, :], 0.0)
            nc.vector.memset(haug[h:h + 1, :], 1.0)
            gi_n = const.tile([h, t_steps], F32, tag="gin")
            ps_b = ps1.tile([h, t_steps], F32, tag="psgb")
            nc.tensor.matmul(ps_b[:], wih[:, 2 * h:3 * h], xaug,
                             start=True, stop=True)
            nc.vector.tensor_copy(gi_n[:], ps_b[:])

            for t in range(t_steps):
                ps_rz = pspool.tile([2 * h, 1], F32, tag="psrz")
                nc.tensor.matmul(ps_rz[:], whh[:, 0:2 * h], haug[:],
                                 start=True, stop=False)
                nc.tensor.matmul(ps_rz[:], wih[:, 0:2 * h], xaug[:, t:t + 1],
                                 start=False, stop=True)
                ps_n = pspool.tile([h, 1], F32, tag="psn")
                nc.tensor.matmul(ps_n[:], whh[:, 2 * h:3 * h], haug[:],
                                 start=True, stop=True)
                sig = sm.tile([2 * h, 1], F32, tag="sig")
                nc.scalar.activation(sig[:], ps_rz[:], AF.Sigmoid)
                n_t = sm.tile([h, 1], F32, tag="nt")
                nc.scalar.activation(n_t[:], ps_n[:], AF.Tanh,
                                     bias=gi_n[:, t:t + 1], scale=sig[0:h, 0:1])
                hmn = sm.tile([h, 1], F32, tag="hmn")
                nc.scalar.activation(hmn[:], n_t[:], AF.Identity,
                                     bias=haug[0:h, 0:1], scale=-1.0)
                nc.scalar.activation(haug[0:h, :], hmn[:], AF.Identity,
                                     bias=n_t[:], scale=sig[h:2 * h, 0:1])

            ps_o = ps1.tile([8, 1], F32, tag="pso")
            nc.tensor.matmul(ps_o[:], hw, haug[:], start=True, stop=True)
            o = sm.tile([8, 1], F32, tag="o")
            nc.scalar.activation(o[:], ps_o[:], AF.Sigmoid)
            nc.sync.dma_start(out_d[:], o[:])
    nc.compile()
    return nc


_P1_CACHE = {}
_P2_CACHE = {}
TRACE = False
LAST_RES = {}
STAGE = 99


def _get_phase1(plan):
    key = plan.key() + (STAGE,)
    if key not in _P1_CACHE:
        nc = bacc.Bacc("TRN2", target_bir_lowering=False, debug=False,
                       num_devices=T)
        _P1_CACHE[key] = build_phase1(nc, plan, stage=STAGE)
    return _P1_CACHE[key]


def _get_phase2():
    if "p2" not in _P2_CACHE:
        nc = bacc.Bacc("TRN2", target_bir_lowering=False, debug=False,
                       num_devices=1)
        _P2_CACHE["p2"] = build_phase2(nc)
    return _P2_CACHE["p2"]


def kernel(x, edge_index, edge_weight, W1, b1, W2, b2, Wih, Whh, bih, bhh,
           headW, headb):
    x = np.asarray(x, np.float32)
    ei = np.asarray(edge_index)
    ew = np.asarray(edge_weight, np.float32)

    preps = [graph_prep(x[t], ei[t, 0].astype(np.int64),
                        ei[t, 1].astype(np.int64), ew[t]) for t in range(T)]
    plan = make_plan(preps)
    nc1 = _get_phase1(plan)
    in_maps = [{"pk": pack_graph(plan, preps[t], W1, W2, b1, b2)}
               for t in range(T)]
    res1 = bass_utils.run_bass_kernel_spmd(nc1, in_maps,
                                           core_ids=list(range(T)),
                                           trace=TRACE)
    LAST_RES["p1"] = res1
    seq = np.stack([np.asarray(res1.results[t]["india"]).reshape(H)
                    for t in range(T)])  # [T, H]

    nc2 = _get_phase2()
    wih_a = np.concatenate([np.asarray(Wih, np.float32).T,
                            np.asarray(bih, np.float32)[None, :]], axis=0)
    whh_a = np.concatenate([np.asarray(Whh, np.float32).T,
                            np.asarray(bhh, np.float32)[None, :]], axis=0)
    hw_a = np.concatenate([np.asarray(headW, np.float32).T,
                           np.asarray(headb, np.float32)[None, :]], axis=0)
    xaug = np.concatenate([seq.T, np.ones((1, T), np.float32)], axis=0)
    fblk = np.concatenate([wih_a, whh_a, hw_a, xaug], axis=1).astype(BF)
    blob2 = np.zeros((P, fblk.shape[1] * 2), np.uint8)
    blob2[0:H + 1] = np.ascontiguousarray(fblk).view(np.uint8)
    res2 = bass_utils.run_bass_kernel_spmd(nc2, [{"pk2": blob2}],
                                           core_ids=[0], trace=TRACE)
    LAST_RES["p2"] = res2
    return np.asarray(res2.results[0]["out"]).reshape(8).astype(np.float32)


# revision 3
# speedup vs baseline: 1.0077x; 1.0077x over previous
"""Trainium2 Bass kernel for nn_SanctionImpactGNN (2-hop pruned).

The model output is sigmoid(heads(GRU(seq))) where seq[t] = h2[node0] of a
2-layer GCN on graph t.  h2[node0] only depends on the 2-hop in-neighborhood
of node 0 (~350 edges of 320000), plus the weighted in-degrees (for the
symmetric normalization) of the ~350 involved nodes.

Host (indexing/packing only -- no FP math):
  * L1 = in-neighbors of node 0 (plus node 0 itself); layer-1 edge slots =
    all edges into L1 nodes + one self-loop slot per L1 node.
  * Per slot: the source node's x row (bf16 column of xT), in-edge weight
    lists of the slot's src and dst stacked along the partition (K) dim so a
    single matmul against a block-ones matrix computes all degree sums, the
    slot's own edge weight, and a one-hot dst indicator column.
  Everything is packed in ONE uint8 blob per graph -> one DMA per core.

Device phase 1 (SPMD, one graph per core):
  degs  = three tiny PE matmuls (lists^T @ block-ones) -> PSUM [128, 3nch+3]
  dis   = reciprocal(sqrt(1+degs))          (one ACT + one DVE op)
  norm  = dis[src]*dis[dst]*ew  per slot    (two DVE ops)
  msg_c = x[src]@W1 per slot chunk (bf16 matmul), copied to SBUF early
  psh1  = b1 + sum_c (A_c * norm_c)^T @ msg_c   (PSUM accumulation; b1 via
          ones-row x b1-row matmul)
  h1    = relu(psh1)                        (DVE tensor_relu)
  india = relu(b2 + dis0 * (W2^T @ (h1^T @ (disL1*s0))))  -> [64,1] f32

Device phase 2 (single core): GRU + heads (bf16 weights/h), per step:
  ps_rz = Whh_rz^T@h + Wih_rz^T@x_t   (PSUM accumulation, bias rows folded)
  sigmoid(ps_rz); tanh(ps_n*r + gi_n); h' = z*(h-n) + n
as 4 activation instructions + 3 tiny matmuls.
"""

import numpy as np
import ml_dtypes

import concourse.bacc as bacc
import concourse.mybir as mybir
import concourse.tile as tile
from concourse import bass_utils

F32 = mybir.dt.float32
BF16 = mybir.dt.bfloat16
U8 = mybir.dt.uint8
AF = mybir.ActivationFunctionType
OP = mybir.AluOpType
AX = mybir.AxisListType

T, N, E, F, H = 8, 20000, 320000, 128, 64
P = 128
INDIA = 0

BF = np.dtype(ml_dtypes.bfloat16)


def _align4(x):
    return (x + 3) & ~3


class Plan:
    """Compile-time sizes shared by all graphs (SPMD)."""

    def __init__(self, nch, nl1, k):
        self.nch, self.nl1, self.k = nch, nl1, k
        self.s1p = nch * P
        # degree-matmul grouping: gpm segments of length k per matmul (<=128
        # partitions each).  Segments: nch src + nch dst + degL1 + deg0 + s0.
        self.gpm = max(1, P // k)
        segs = 2 * nch + 3
        self.nmm = (segs + self.gpm - 1) // self.gpm
        self.segs = segs
        o = 0
        self.o_xt = o; o += self.s1p * 2                 # bf16 [128, S1P]
        self.o_dl = o; o += self.nmm * P * 2             # bf16 [gpm*k, nmm*128]
        self.o_on = o; o += self.gpm * self.segs * 2     # bf16 ones blocks
        self.o_a = o; o = _align4(o + nch * nl1 * 2)     # bf16 [128, nch*NL1]
        self.o_ew = o; o += nch * 4                      # f32  [128, nch]
        self.o_w1 = o; o += H * 2                        # bf16 [128, 64]
        self.o_w2 = o; o += H * 2                        # bf16 [64, 64]
        self.o_b1 = o; o += H * 2                        # bf16 [1, 64] row
        self.o_or = o; o = _align4(o + 2)                # bf16 [1, 1] one
        self.o_b2 = o; o = _align4(o + 4)                # f32  [64, 1]
        self.pb = _align4(o)

    def key(self):
        return (self.nch, self.nl1, self.k)


def graph_prep(x_t, src, dst, ew):
    """Per-graph host extraction (pure indexing).  Returns raw structures."""
    order = np.argsort(dst, kind="stable")
    dst_s = dst[order]
    starts = np.searchsorted(dst_s, np.arange(N + 1))

    def in_edges(v):          # edge ids (original) into node v
        return order[starts[v]:starts[v + 1]]

    e0 = in_edges(INDIA)
    l1rest = np.unique(src[e0])
    l1rest = l1rest[l1rest != INDIA]
    L1 = np.concatenate([[INDIA], l1rest]).astype(np.int64)
    nl1 = len(L1)

    slot_src, slot_dst, slot_ew = [], [], []
    for j, v in enumerate(L1):
        eids = in_edges(v)
        slot_src.append(src[eids])
        slot_dst.append(np.full(len(eids) + 1, j, np.int64))
        slot_ew.append(ew[eids])
        # self-loop
        slot_src.append(np.array([v], np.int64))
        slot_ew.append(np.array([1.0], np.float32))
    slot_src = np.concatenate(slot_src).astype(np.int64)
    slot_dst = np.concatenate(slot_dst)
    slot_ew = np.concatenate(slot_ew).astype(np.float32)
    s1 = len(slot_src)

    involved = np.unique(np.concatenate([slot_src, L1]))
    indeg = starts[1:] - starts[:-1]
    kmax = int(indeg[involved].max()) if len(involved) else 1
    # s0 lists: edges L1[j] -> 0 (+ self-loop 1.0 for j==0)
    s0_lists = []
    src_e0 = src[e0]
    for j, v in enumerate(L1):
        vals = ew[e0[src_e0 == v]]
        if v == INDIA:
            vals = np.concatenate([vals, [np.float32(1.0)]])
        s0_lists.append(np.asarray(vals, np.float32))
    kmax = max(kmax, max(len(v) for v in s0_lists))
    return dict(L1=L1, nl1=nl1, slot_src=slot_src, slot_dst=slot_dst,
                slot_ew=slot_ew, s1=s1, kmax=kmax, s0_lists=s0_lists,
                in_edges=in_edges, ew=ew, x_t=x_t)


def make_plan(preps):
    s1 = max(p["s1"] for p in preps)
    nch = (s1 + P - 1) // P
    nl1 = max(p["nl1"] for p in preps)
    k = max(p["kmax"] for p in preps)
    return Plan(nch, nl1, k)


def pack_graph(plan, prep, W1, W2, b1, b2):
    nch, nl1, k = plan.nch, plan.nl1, plan.k
    gpm, segs, nmm = plan.gpm, plan.segs, plan.nmm
    s1p = plan.s1p
    blob = np.zeros((P, plan.pb), np.uint8)

    def put(off, arr, rows=P):
        a = np.ascontiguousarray(arr)
        b = a.view(np.uint8).reshape(rows, -1)
        blob[:rows, off:off + b.shape[1]] = b

    ns = prep["s1"]
    ssrc, sdst, sew = prep["slot_src"], prep["slot_dst"], prep["slot_ew"]
    in_edges, ew = prep["in_edges"], prep["ew"]
    L1 = prep["L1"]

    # xT: [128 features, S1P slots] bf16
    xt = np.zeros((P, s1p), BF)
    xt[:, :ns] = np.asarray(prep["x_t"], np.float32)[ssrc].T.astype(BF)
    put(plan.o_xt, xt)

    # degree list segments, each [k, 128] (cols = lanes):
    #   seg c       (c<nch):    in-ew list of src(slot c,p) at col p
    #   seg nch+c:              in-ew list of dst(slot c,p) at col p
    #   seg 2nch:               in-ew list of L1[p]  (cols < nl1)
    #   seg 2nch+1:             in-ew list of node 0 (all 64 cols)
    #   seg 2nch+2:             ew of edges L1[p] -> 0 (+1.0 at p==0)
    seglists = np.zeros((segs, k, P), BF)
    lane, chunk = np.arange(ns) % P, np.arange(ns) // P
    l1arr = L1[sdst]
    for s in range(ns):
        lst = ew[in_edges(ssrc[s])]
        seglists[chunk[s], :len(lst), lane[s]] = lst.astype(BF)
        lstd = ew[in_edges(l1arr[s])]
        seglists[nch + chunk[s], :len(lstd), lane[s]] = lstd.astype(BF)
    for j, v in enumerate(L1):
        lst = ew[in_edges(v)]
        seglists[2 * nch, :len(lst), j] = lst.astype(BF)
        s0l = prep["s0_lists"][j]
        seglists[2 * nch + 2, :len(s0l), j] = s0l.astype(BF)
    l0 = ew[in_edges(INDIA)].astype(BF)
    seglists[2 * nch + 1, :len(l0), :H] = l0[:, None]

    # stack gpm segments per matmul along partitions; ones blocks map each
    # K-rows group to its psum column.
    dlm = np.zeros((P, nmm * P), BF)
    onem = np.zeros((P, gpm * segs), BF)
    for si in range(segs):
        m, g = si // gpm, si % gpm
        dlm[g * k:(g + 1) * k, m * P:(m + 1) * P] = seglists[si]
        onem[g * k:(g + 1) * k, si] = np.float32(1.0)
    put(plan.o_dl, dlm)
    put(plan.o_on, onem)

    A = np.zeros((P, nch, nl1), BF)
    ewm = np.zeros((P, nch), np.float32)
    ewm[lane, chunk] = sew
    A[lane, chunk, sdst] = np.float32(1.0)
    put(plan.o_a, A.reshape(P, -1))
    put(plan.o_ew, ewm)

    put(plan.o_w1, np.asarray(W1, np.float32).astype(BF))            # [128,64]
    put(plan.o_w2, np.asarray(W2, np.float32).astype(BF), rows=H)    # [64,64]
    put(plan.o_b1, np.asarray(b1, np.float32).astype(BF).reshape(1, H), rows=1)
    put(plan.o_or, np.ones((1, 1), BF), rows=1)
    put(plan.o_b2, np.asarray(b2, np.float32).reshape(H, 1), rows=H)
    return blob


def build_phase1(nc, plan, stage=99):
    nch, nl1, k = plan.nch, plan.nl1, plan.k
    gpm, segs, nmm = plan.gpm, plan.segs, plan.nmm
    pk_d = nc.dram_tensor("pk", [P, plan.pb], U8, kind="ExternalInput")
    india_d = nc.dram_tensor("india", [H, 1], F32, kind="ExternalOutput")
    dbg_d = None
    if stage < 99:
        dbg_d = nc.dram_tensor("dbg", [P, 256], F32, kind="ExternalOutput")

    with tile.TileContext(nc) as tc:
        with (
            tc.tile_pool(name="const", bufs=1) as const,
            tc.tile_pool(name="sm", bufs=1) as sm,
            tc.tile_pool(name="wm", bufs=3) as wm,
            tc.tile_pool(name="psm", bufs=3, space="PSUM") as psmp,
            tc.tile_pool(name="ps1", bufs=1, space="PSUM") as ps1,
        ):
            pk = const.tile([P, plan.pb], U8, tag="pk")
            nc.sync.dma_start(pk[:], pk_d[:])

            xv = pk[:, plan.o_xt:plan.o_xt + plan.s1p * 2].bitcast(BF16)
            dlv = pk[:, plan.o_dl:plan.o_dl + nmm * P * 2].bitcast(BF16)
            onv = pk[:, plan.o_on:plan.o_on + gpm * segs * 2].bitcast(BF16)
            av = pk[:, plan.o_a:plan.o_a + nch * nl1 * 2].bitcast(BF16)
            ewv = pk[:, plan.o_ew:plan.o_ew + nch * 4].bitcast(F32)
            w1v = pk[:, plan.o_w1:plan.o_w1 + H * 2].bitcast(BF16)
            w2v = pk[0:H, plan.o_w2:plan.o_w2 + H * 2].bitcast(BF16)
            b1v = pk[0:1, plan.o_b1:plan.o_b1 + H * 2].bitcast(BF16)
            onr = pk[0:1, plan.o_or:plan.o_or + 2].bitcast(BF16)
            b2v = pk[0:H, plan.o_b2:plan.o_b2 + 4].bitcast(F32)

            # --- degree sums via block-ones matmuls -> psd [128, segs]
            psd = ps1.tile([P, segs], F32, tag="psd")
            for m in range(nmm):
                lo, hi = m * gpm, min((m + 1) * gpm, segs)
                nsg = hi - lo
                nc.tensor.matmul(psd[:, lo:hi],
                                 dlv[0:gpm * k, m * P:(m + 1) * P],
                                 onv[0:gpm * k, lo:hi],
                                 start=True, stop=True)
            # dis = 1/sqrt(1+deg) for all but the raw-s0 column
            dsq = sm.tile([P, segs - 1], F32, tag="dsq")
            nc.scalar.activation(dsq[:], psd[:, 0:segs - 1], AF.Sqrt, bias=1.0)
            dis = sm.tile([P, segs - 1], F32, tag="dis")
            nc.vector.reciprocal(dis[:], dsq[:])
            # norm = dis_src * dis_dst * ew  [128, nch]
            norm = sm.tile([P, nch], F32, tag="norm")
            nc.vector.tensor_mul(norm[:], dis[:, 0:nch], dis[:, nch:2 * nch])
            nc.vector.tensor_mul(norm[:], norm[:], ewv)
            # cvec = disL1 * s0_raw (rows < nl1)
            cvec = sm.tile([H, 1], BF16, tag="cvec")
            nc.vector.tensor_mul(cvec[:], dis[0:H, 2 * nch:2 * nch + 1],
                                 psd[0:H, segs - 1:segs])

            if stage == 0:
                z = sm.tile([P, 256], F32, tag="dbgt")
                nc.vector.memset(z[:], 0.0)
                nc.vector.tensor_copy(z[:, 0:segs], psd[:])
                nc.vector.tensor_copy(z[:, 16:16 + nch], norm[:])
                nc.vector.tensor_copy(z[0:H, 24:25], cvec[:])
                nc.sync.dma_start(dbg_d[:], z[:])

            # --- per-chunk messages; copies off critical path
            psh1 = ps1.tile([nl1, H], F32, tag="psh1")
            # b1 enters the accumulation first (no data deps beyond blob)
            nc.tensor.matmul(psh1[:], onr.broadcast_to((1, nl1)), b1v,
                             start=True, stop=False)
            msgs = []
            for c in range(nch):
                psm = psmp.tile([P, H], F32, tag="psm")
                nc.tensor.matmul(psm[:], xv[:, c * P:(c + 1) * P], w1v,
                                 start=True, stop=True)
                msg = wm.tile([P, H], BF16, tag="msg")
                if c == 0:
                    nc.vector.tensor_copy(msg[:], psm[:])
                else:
                    nc.scalar.copy(msg[:], psm[:])
                msgs.append(msg)
            asc = sm.tile([P, nch * nl1], BF16, tag="asc")
            for c in range(nch):
                nc.vector.tensor_scalar_mul(
                    asc[:, c * nl1:(c + 1) * nl1],
                    av[:, c * nl1:(c + 1) * nl1], norm[:, c:c + 1])
                nc.tensor.matmul(psh1[:], asc[:, c * nl1:(c + 1) * nl1],
                                 msgs[c][:], start=False, stop=(c == nch - 1))

            h1 = sm.tile([nl1, H], BF16, tag="h1")
            nc.vector.tensor_relu(h1[:], psh1[:])

            if stage == 1:
                z = sm.tile([P, 256], F32, tag="dbgt")
                nc.vector.memset(z[:], 0.0)
                nc.vector.tensor_copy(z[0:nl1, 0:H], psh1[:])
                nc.sync.dma_start(dbg_d[:], z[:])

            ps_a = ps1.tile([H, 1], F32, tag="psa")
            nc.tensor.matmul(ps_a[:], h1[:], cvec[0:nl1, 0:1],
                             start=True, stop=True)
            agg1 = sm.tile([H, 1], BF16, tag="agg1")
            nc.vector.tensor_copy(agg1[:], ps_a[:])
            ps_h2 = ps1.tile([H, 1], F32, tag="psh2")
            nc.tensor.matmul(ps_h2[:], w2v, agg1[:], start=True, stop=True)
            india = sm.tile([H, 1], F32, tag="india")
            nc.scalar.activation(india[:], ps_h2[:], AF.Relu,
                                 scale=dis[0:H, 2 * nch + 1:2 * nch + 2],
                                 bias=b2v)
            nc.sync.dma_start(india_d[:], india[:])
    nc.compile()
    return nc


def build_phase2(nc):
    h, t_steps = H, T
    # packed bf16 blob rows 0..64: wihT_aug | whhT_aug | headWT_aug | xaug | h0
    w2cols = 3 * h + 3 * h + 8 + t_steps + 1
    pk_d = nc.dram_tensor("pk2", [P, w2cols * 2], U8, kind="ExternalInput")
    out_d = nc.dram_tensor("out", [8, 1], F32, kind="ExternalOutput")

    with tile.TileContext(nc) as tc:
        with (
            tc.tile_pool(name="const", bufs=1) as const,
            tc.tile_pool(name="sm", bufs=4) as sm,
            tc.tile_pool(name="ps", bufs=2, space="PSUM") as pspool,
            tc.tile_pool(name="ps1", bufs=1, space="PSUM") as ps1,
        ):
            pk = const.tile([P, w2cols * 2], U8, tag="pk2")
            nc.sync.dma_start(pk[:], pk_d[:])
            fv = pk[0:h + 1, :].bitcast(BF16)
            wih = fv[:, 0:3 * h]
            whh = fv[:, 3 * h:6 * h]
            hw = fv[:, 6 * h:6 * h + 8]
            xaug = fv[:, 6 * h + 8:6 * h + 8 + t_steps]

            # h lives in the blob: initial value [0]*64 + [1] is loaded with
            # the weights; the per-step update overwrites rows 0..63 in place.
            haug = fv[:, 6 * h + 8 + t_steps:6 * h + 9 + t_steps]
            gi_n = const.tile([h, t_steps], F32, tag="gin")
            ps_b = ps1.tile([h, t_steps], F32, tag="psgb")
            nc.tensor.matmul(ps_b[:], wih[:, 2 * h:3 * h], xaug,
                             start=True, stop=True)
            nc.vector.tensor_copy(gi_n[:], ps_b[:])

            for t in range(t_steps):
                ps_rz = pspool.tile([2 * h, 1], F32, tag="psrz")
                nc.tensor.matmul(ps_rz[:], whh[:, 0:2 * h], haug[:],
                                 start=True, stop=False)
                nc.tensor.matmul(ps_rz[:], wih[:, 0:2 * h], xaug[:, t:t + 1],
                                 start=False, stop=True)
                ps_n = pspool.tile([h, 1], F32, tag="psn")
                nc.tensor.matmul(ps_n[:], whh[:, 2 * h:3 * h], haug[:],
                                 start=True, stop=True)
                sig = sm.tile([2 * h, 1], F32, tag="sig")
                nc.scalar.activation(sig[:], ps_rz[:], AF.Sigmoid)
                n_t = sm.tile([h, 1], F32, tag="nt")
                nc.scalar.activation(n_t[:], ps_n[:], AF.Tanh,
                                     bias=gi_n[:, t:t + 1], scale=sig[0:h, 0:1])
                hmn = sm.tile([h, 1], F32, tag="hmn")
                nc.scalar.activation(hmn[:], n_t[:], AF.Identity,
                                     bias=haug[0:h, 0:1], scale=-1.0)
                nc.scalar.activation(haug[0:h, :], hmn[:], AF.Identity,
                                     bias=n_t[:], scale=sig[h:2 * h, 0:1])

            ps_o = ps1.tile([8, 1], F32, tag="pso")
            nc.tensor.matmul(ps_o[:], hw, haug[:], start=True, stop=True)
            o = sm.tile([8, 1], F32, tag="o")
            nc.scalar.activation(o[:], ps_o[:], AF.Sigmoid)
            nc.sync.dma_start(out_d[:], o[:])
    nc.compile()
    return nc


_P1_CACHE = {}
_P2_CACHE = {}
TRACE = False
LAST_RES = {}
STAGE = 99


def _get_phase1(plan):
    key = plan.key() + (STAGE,)
    if key not in _P1_CACHE:
        nc = bacc.Bacc("TRN2", target_bir_lowering=False, debug=False,
                       num_devices=T)
        _P1_CACHE[key] = build_phase1(nc, plan, stage=STAGE)
    return _P1_CACHE[key]


def _get_phase2():
    if "p2" not in _P2_CACHE:
        nc = bacc.Bacc("TRN2", target_bir_lowering=False, debug=False,
                       num_devices=1)
        _P2_CACHE["p2"] = build_phase2(nc)
    return _P2_CACHE["p2"]


def kernel(x, edge_index, edge_weight, W1, b1, W2, b2, Wih, Whh, bih, bhh,
           headW, headb):
    x = np.asarray(x, np.float32)
    ei = np.asarray(edge_index)
    ew = np.asarray(edge_weight, np.float32)

    preps = [graph_prep(x[t], ei[t, 0].astype(np.int64),
                        ei[t, 1].astype(np.int64), ew[t]) for t in range(T)]
    plan = make_plan(preps)
    nc1 = _get_phase1(plan)
    in_maps = [{"pk": pack_graph(plan, preps[t], W1, W2, b1, b2)}
               for t in range(T)]
    res1 = bass_utils.run_bass_kernel_spmd(nc1, in_maps,
                                           core_ids=list(range(T)),
                                           trace=TRACE)
    LAST_RES["p1"] = res1
    seq = np.stack([np.asarray(res1.results[t]["india"]).reshape(H)
                    for t in range(T)])  # [T, H]

    nc2 = _get_phase2()
    wih_a = np.concatenate([np.asarray(Wih, np.float32).T,
                            np.asarray(bih, np.float32)[None, :]], axis=0)
    whh_a = np.concatenate([np.asarray(Whh, np.float32).T,
                            np.asarray(bhh, np.float32)[None, :]], axis=0)
    hw_a = np.concatenate([np.asarray(headW, np.float32).T,
                           np.asarray(headb, np.float32)[None, :]], axis=0)
    xaug = np.concatenate([seq.T, np.ones((1, T), np.float32)], axis=0)
    h0 = np.zeros((H + 1, 1), np.float32)
    h0[H, 0] = 1.0
    fblk = np.concatenate([wih_a, whh_a, hw_a, xaug, h0], axis=1).astype(BF)
    blob2 = np.zeros((P, fblk.shape[1] * 2), np.uint8)
    blob2[0:H + 1] = np.ascontiguousarray(fblk).view(np.uint8)
    res2 = bass_utils.run_bass_kernel_spmd(nc2, [{"pk2": blob2}],
                                           core_ids=[0], trace=TRACE)
    LAST_RES["p2"] = res2
    return np.asarray(res2.results[0]["out"]).reshape(8).astype(np.float32)


# revision 4
# speedup vs baseline: 1.0106x; 1.0029x over previous
"""Trainium2 Bass kernel for nn_SanctionImpactGNN (2-hop pruned).

The model output is sigmoid(heads(GRU(seq))) where seq[t] = h2[node0] of a
2-layer GCN on graph t.  h2[node0] only depends on the 2-hop in-neighborhood
of node 0 (~350 edges of 320000), plus the weighted in-degrees (for the
symmetric normalization) of the ~350 involved nodes.

Host (indexing/packing only -- no FP math):
  * L1 = in-neighbors of node 0 (plus node 0 itself); layer-1 edge slots =
    all edges into L1 nodes + one self-loop slot per L1 node.
  * Per slot: the source node's x row (bf16 column of xT), in-edge weight
    lists of the slot's src and dst stacked along the partition (K) dim so a
    single matmul against a block-ones matrix computes all degree sums, the
    slot's own edge weight, and a one-hot dst indicator column.
  Everything is packed in ONE uint8 blob per graph -> one DMA per core.

Device phase 1 (SPMD, one graph per core):
  degs  = three tiny PE matmuls (lists^T @ block-ones) -> PSUM [128, 3nch+3]
  dis   = reciprocal(sqrt(1+degs))          (one ACT + one DVE op)
  norm  = dis[src]*dis[dst]*ew  per slot    (two DVE ops)
  msg_c = x[src]@W1 per slot chunk (bf16 matmul), copied to SBUF early
  psh1  = b1 + sum_c (A_c * norm_c)^T @ msg_c   (PSUM accumulation; b1 via
          ones-row x b1-row matmul)
  h1    = relu(psh1)                        (DVE tensor_relu)
  india = relu(b2 + dis0 * (W2^T @ (h1^T @ (disL1*s0))))  -> [64,1] f32

Device phase 2 (single core): GRU + heads (bf16 weights/h), per step:
  ps_rz = Whh_rz^T@h + Wih_rz^T@x_t   (PSUM accumulation, bias rows folded)
  sigmoid(ps_rz); tanh(ps_n*r + gi_n); h' = z*(h-n) + n
as 4 activation instructions + 3 tiny matmuls.
"""

import numpy as np
import ml_dtypes

import concourse.bacc as bacc
import concourse.mybir as mybir
import concourse.tile as tile
from concourse import bass_utils

F32 = mybir.dt.float32
BF16 = mybir.dt.bfloat16
U8 = mybir.dt.uint8
AF = mybir.ActivationFunctionType
OP = mybir.AluOpType
AX = mybir.AxisListType

T, N, E, F, H = 8, 20000, 320000, 128, 64
P = 128
INDIA = 0

BF = np.dtype(ml_dtypes.bfloat16)


def _align4(x):
    return (x + 3) & ~3


class Plan:
    """Compile-time sizes shared by all graphs (SPMD)."""

    def __init__(self, nch, nl1, k):
        self.nch, self.nl1, self.k = nch, nl1, k
        self.s1p = nch * P
        # degree-matmul grouping: gpm segments of length k per matmul (<=128
        # partitions each).  Segments: nch src + nch dst + degL1 + deg0 + s0.
        self.gpm = max(1, P // k)
        segs = 2 * nch + 3
        self.nmm = (segs + self.gpm - 1) // self.gpm
        self.segs = segs
        o = 0
        self.o_dl = o; o += self.nmm * P * 2             # bf16 [gpm*k, nmm*128]
        self.o_on = o; o += self.gpm * self.segs * 2     # bf16 ones blocks
        self.o_a = o; o = _align4(o + nch * nl1 * 2)     # bf16 [128, nch*NL1]
        self.o_ew = o; o += nch * 4                      # f32  [128, nch]
        self.o_w1 = o; o += H * 2                        # bf16 [128, 64]
        self.o_w2 = o; o += H * 2                        # bf16 [64, 64]
        self.o_b1 = o; o += H * 2                        # bf16 [1, 64] row
        self.o_or = o; o = _align4(o + 2)                # bf16 [1, 1] one
        self.o_b2 = o; o = _align4(o + 4)                # f32  [64, 1]
        self.pb = _align4(o)
        self.pbx = self.s1p * 2                          # bf16 [128, S1P] xT

    def key(self):
        return (self.nch, self.nl1, self.k)


def graph_prep(x_t, src, dst, ew):
    """Per-graph host extraction (pure indexing).  Returns raw structures."""
    order = np.argsort(dst, kind="stable")
    dst_s = dst[order]
    starts = np.searchsorted(dst_s, np.arange(N + 1))

    def in_edges(v):          # edge ids (original) into node v
        return order[starts[v]:starts[v + 1]]

    e0 = in_edges(INDIA)
    l1rest = np.unique(src[e0])
    l1rest = l1rest[l1rest != INDIA]
    L1 = np.concatenate([[INDIA], l1rest]).astype(np.int64)
    nl1 = len(L1)

    slot_src, slot_dst, slot_ew = [], [], []
    for j, v in enumerate(L1):
        eids = in_edges(v)
        slot_src.append(src[eids])
        slot_dst.append(np.full(len(eids) + 1, j, np.int64))
        slot_ew.append(ew[eids])
        # self-loop
        slot_src.append(np.array([v], np.int64))
        slot_ew.append(np.array([1.0], np.float32))
    slot_src = np.concatenate(slot_src).astype(np.int64)
    slot_dst = np.concatenate(slot_dst)
    slot_ew = np.concatenate(slot_ew).astype(np.float32)
    s1 = len(slot_src)

    involved = np.unique(np.concatenate([slot_src, L1]))
    indeg = starts[1:] - starts[:-1]
    kmax = int(indeg[involved].max()) if len(involved) else 1
    # s0 lists: edges L1[j] -> 0 (+ self-loop 1.0 for j==0)
    s0_lists = []
    src_e0 = src[e0]
    for j, v in enumerate(L1):
        vals = ew[e0[src_e0 == v]]
        if v == INDIA:
            vals = np.concatenate([vals, [np.float32(1.0)]])
        s0_lists.append(np.asarray(vals, np.float32))
    kmax = max(kmax, max(len(v) for v in s0_lists))
    return dict(L1=L1, nl1=nl1, slot_src=slot_src, slot_dst=slot_dst,
                slot_ew=slot_ew, s1=s1, kmax=kmax, s0_lists=s0_lists,
                in_edges=in_edges, ew=ew, x_t=x_t)


def make_plan(preps):
    s1 = max(p["s1"] for p in preps)
    nch = (s1 + P - 1) // P
    nl1 = max(p["nl1"] for p in preps)
    k = max(p["kmax"] for p in preps)
    return Plan(nch, nl1, k)


def pack_graph(plan, prep, W1, W2, b1, b2):
    nch, nl1, k = plan.nch, plan.nl1, plan.k
    gpm, segs, nmm = plan.gpm, plan.segs, plan.nmm
    s1p = plan.s1p
    blob = np.zeros((P, plan.pb), np.uint8)
    blobx = np.zeros((P, plan.pbx), np.uint8)

    def put(off, arr, rows=P):
        a = np.ascontiguousarray(arr)
        b = a.view(np.uint8).reshape(rows, -1)
        blob[:rows, off:off + b.shape[1]] = b

    ns = prep["s1"]
    ssrc, sdst, sew = prep["slot_src"], prep["slot_dst"], prep["slot_ew"]
    in_edges, ew = prep["in_edges"], prep["ew"]
    L1 = prep["L1"]

    # xT: [128 features, S1P slots] bf16 -> its own blob (second DMA)
    xt = np.zeros((P, s1p), BF)
    xt[:, :ns] = np.asarray(prep["x_t"], np.float32)[ssrc].T.astype(BF)
    blobx[:] = np.ascontiguousarray(xt).view(np.uint8)

    # degree list segments, each [k, 128] (cols = lanes):
    #   seg c       (c<nch):    in-ew list of src(slot c,p) at col p
    #   seg nch+c:              in-ew list of dst(slot c,p) at col p
    #   seg 2nch:               in-ew list of L1[p]  (cols < nl1)
    #   seg 2nch+1:             in-ew list of node 0 (all 64 cols)
    #   seg 2nch+2:             ew of edges L1[p] -> 0 (+1.0 at p==0)
    seglists = np.zeros((segs, k, P), BF)
    lane, chunk = np.arange(ns) % P, np.arange(ns) // P
    l1arr = L1[sdst]
    for s in range(ns):
        lst = ew[in_edges(ssrc[s])]
        seglists[chunk[s], :len(lst), lane[s]] = lst.astype(BF)
        lstd = ew[in_edges(l1arr[s])]
        seglists[nch + chunk[s], :len(lstd), lane[s]] = lstd.astype(BF)
    for j, v in enumerate(L1):
        lst = ew[in_edges(v)]
        seglists[2 * nch, :len(lst), j] = lst.astype(BF)
        s0l = prep["s0_lists"][j]
        seglists[2 * nch + 2, :len(s0l), j] = s0l.astype(BF)
    l0 = ew[in_edges(INDIA)].astype(BF)
    seglists[2 * nch + 1, :len(l0), :H] = l0[:, None]

    # stack gpm segments per matmul along partitions; ones blocks map each
    # K-rows group to its psum column.
    dlm = np.zeros((P, nmm * P), BF)
    onem = np.zeros((P, gpm * segs), BF)
    for si in range(segs):
        m, g = si // gpm, si % gpm
        dlm[g * k:(g + 1) * k, m * P:(m + 1) * P] = seglists[si]
        onem[g * k:(g + 1) * k, si] = np.float32(1.0)
    put(plan.o_dl, dlm)
    put(plan.o_on, onem)

    A = np.zeros((P, nch, nl1), BF)
    ewm = np.zeros((P, nch), np.float32)
    ewm[lane, chunk] = sew
    A[lane, chunk, sdst] = np.float32(1.0)
    put(plan.o_a, A.reshape(P, -1))
    put(plan.o_ew, ewm)

    put(plan.o_w1, np.asarray(W1, np.float32).astype(BF))            # [128,64]
    put(plan.o_w2, np.asarray(W2, np.float32).astype(BF), rows=H)    # [64,64]
    put(plan.o_b1, np.asarray(b1, np.float32).astype(BF).reshape(1, H), rows=1)
    put(plan.o_or, np.ones((1, 1), BF), rows=1)
    put(plan.o_b2, np.asarray(b2, np.float32).reshape(H, 1), rows=H)
    return blob, blobx


def build_phase1(nc, plan, stage=99):
    nch, nl1, k = plan.nch, plan.nl1, plan.k
    gpm, segs, nmm = plan.gpm, plan.segs, plan.nmm
    pk_d = nc.dram_tensor("pk", [P, plan.pb], U8, kind="ExternalInput")
    pkx_d = nc.dram_tensor("pkx", [P, plan.pbx], U8, kind="ExternalInput")
    india_d = nc.dram_tensor("india", [H, 1], F32, kind="ExternalOutput")
    dbg_d = None
    if stage < 99:
        dbg_d = nc.dram_tensor("dbg", [P, 256], F32, kind="ExternalOutput")

    with tile.TileContext(nc) as tc:
        with (
            tc.tile_pool(name="const", bufs=1) as const,
            tc.tile_pool(name="sm", bufs=1) as sm,
            tc.tile_pool(name="wm", bufs=3) as wm,
            tc.tile_pool(name="psm", bufs=3, space="PSUM") as psmp,
            tc.tile_pool(name="ps1", bufs=1, space="PSUM") as ps1,
        ):
            pk = const.tile([P, plan.pb], U8, tag="pk")
            nc.sync.dma_start(pk[:], pk_d[:])
            pkx = const.tile([P, plan.pbx], U8, tag="pkx")
            nc.sync.dma_start(pkx[:], pkx_d[:])

            xv = pkx[:, :].bitcast(BF16)
            dlv = pk[:, plan.o_dl:plan.o_dl + nmm * P * 2].bitcast(BF16)
            onv = pk[:, plan.o_on:plan.o_on + gpm * segs * 2].bitcast(BF16)
            av = pk[:, plan.o_a:plan.o_a + nch * nl1 * 2].bitcast(BF16)
            ewv = pk[:, plan.o_ew:plan.o_ew + nch * 4].bitcast(F32)
            w1v = pk[:, plan.o_w1:plan.o_w1 + H * 2].bitcast(BF16)
            w2v = pk[0:H, plan.o_w2:plan.o_w2 + H * 2].bitcast(BF16)
            b1v = pk[0:1, plan.o_b1:plan.o_b1 + H * 2].bitcast(BF16)
            onr = pk[0:1, plan.o_or:plan.o_or + 2].bitcast(BF16)
            b2v = pk[0:H, plan.o_b2:plan.o_b2 + 4].bitcast(F32)

            # --- degree sums via block-ones matmuls -> psd [128, segs]
            psd = ps1.tile([P, segs], F32, tag="psd")
            for m in range(nmm):
                lo, hi = m * gpm, min((m + 1) * gpm, segs)
                nsg = hi - lo
                nc.tensor.matmul(psd[:, lo:hi],
                                 dlv[0:gpm * k, m * P:(m + 1) * P],
                                 onv[0:gpm * k, lo:hi],
                                 start=True, stop=True)
            # dis = 1/sqrt(1+deg) for all but the raw-s0 column
            dsq = sm.tile([P, segs - 1], F32, tag="dsq")
            nc.scalar.activation(dsq[:], psd[:, 0:segs - 1], AF.Sqrt, bias=1.0)
            dis = sm.tile([P, segs - 1], F32, tag="dis")
            nc.vector.reciprocal(dis[:], dsq[:])
            # norm = dis_src * dis_dst * ew  [128, nch]
            norm = sm.tile([P, nch], F32, tag="norm")
            nc.vector.tensor_mul(norm[:], dis[:, 0:nch], dis[:, nch:2 * nch])
            nc.vector.tensor_mul(norm[:], norm[:], ewv)

            if stage == 0:
                z = sm.tile([P, 256], F32, tag="dbgt")
                nc.vector.memset(z[:], 0.0)
                nc.vector.tensor_copy(z[:, 0:segs], psd[:])
                nc.vector.tensor_copy(z[:, 16:16 + nch], norm[:])
                nc.vector.tensor_copy(z[0:H, 24:25], cvec[:])
                nc.sync.dma_start(dbg_d[:], z[:])

            # --- per-chunk messages; copies off critical path
            psh1 = ps1.tile([nl1, H], F32, tag="psh1")
            # b1 enters the accumulation first (no data deps beyond blob)
            nc.tensor.matmul(psh1[:], onr.broadcast_to((1, nl1)), b1v,
                             start=True, stop=False)
            msgs = []
            for c in range(nch):
                psm = psmp.tile([P, H], F32, tag="psm")
                nc.tensor.matmul(psm[:], xv[:, c * P:(c + 1) * P], w1v,
                                 start=True, stop=True)
                msg = wm.tile([P, H], BF16, tag="msg")
                if c == 0:
                    nc.vector.tensor_copy(msg[:], psm[:])
                else:
                    nc.scalar.copy(msg[:], psm[:])
                msgs.append(msg)
            asc = sm.tile([P, nch * nl1], BF16, tag="asc")
            nc.vector.tensor_tensor(
                asc[:].rearrange("p (c v) -> p c v", v=nl1),
                av.rearrange("p (c v) -> p c v", v=nl1),
                norm[:].unsqueeze(2).broadcast_to((P, nch, nl1)),
                op=OP.mult)
            for c in range(nch):
                nc.tensor.matmul(psh1[:], asc[:, c * nl1:(c + 1) * nl1],
                                 msgs[c][:], start=False, stop=(c == nch - 1))
            # cvec = disL1 * s0_raw (rows < nl1); emitted after the ascs so
            # the scheduler prioritizes the norm->asc->agg chain
            cvec = sm.tile([H, 1], BF16, tag="cvec")
            nc.vector.tensor_mul(cvec[:], dis[0:H, 2 * nch:2 * nch + 1],
                                 psd[0:H, segs - 1:segs])

            h1 = sm.tile([nl1, H], BF16, tag="h1")
            nc.vector.tensor_relu(h1[:], psh1[:])

            if stage == 1:
                z = sm.tile([P, 256], F32, tag="dbgt")
                nc.vector.memset(z[:], 0.0)
                nc.vector.tensor_copy(z[0:nl1, 0:H], psh1[:])
                nc.sync.dma_start(dbg_d[:], z[:])

            ps_a = ps1.tile([H, 1], F32, tag="psa")
            nc.tensor.matmul(ps_a[:], h1[:], cvec[0:nl1, 0:1],
                             start=True, stop=True)
            agg1 = sm.tile([H, 1], BF16, tag="agg1")
            nc.vector.tensor_copy(agg1[:], ps_a[:])
            ps_h2 = ps1.tile([H, 1], F32, tag="psh2")
            nc.tensor.matmul(ps_h2[:], w2v, agg1[:], start=True, stop=True)
            india = sm.tile([H, 1], F32, tag="india")
            nc.scalar.activation(india[:], ps_h2[:], AF.Relu,
                                 scale=dis[0:H, 2 * nch + 1:2 * nch + 2],
                                 bias=b2v)
            nc.sync.dma_start(india_d[:], india[:])
    nc.compile()
    return nc


def build_phase2(nc):
    h, t_steps = H, T
    # packed bf16 blob rows 0..64: wihT_aug | whhT_aug | headWT_aug | xaug | h0
    w2cols = 3 * h + 3 * h + 8 + t_steps + 1
    pk_d = nc.dram_tensor("pk2", [P, w2cols * 2], U8, kind="ExternalInput")
    out_d = nc.dram_tensor("out", [8, 1], F32, kind="ExternalOutput")

    with tile.TileContext(nc) as tc:
        with (
            tc.tile_pool(name="const", bufs=1) as const,
            tc.tile_pool(name="sm", bufs=4) as sm,
            tc.tile_pool(name="ps", bufs=2, space="PSUM") as pspool,
            tc.tile_pool(name="ps1", bufs=1, space="PSUM") as ps1,
        ):
            pk = const.tile([P, w2cols * 2], U8, tag="pk2")
            nc.sync.dma_start(pk[:], pk_d[:])
            fv = pk[0:h + 1, :].bitcast(BF16)
            wih = fv[:, 0:3 * h]
            whh = fv[:, 3 * h:6 * h]
            hw = fv[:, 6 * h:6 * h + 8]
            xaug = fv[:, 6 * h + 8:6 * h + 8 + t_steps]

            # h lives in the blob: initial value [0]*64 + [1] is loaded with
            # the weights; the per-step update overwrites rows 0..63 in place.
            haug = fv[:, 6 * h + 8 + t_steps:6 * h + 9 + t_steps]
            gi_n = const.tile([h, t_steps], F32, tag="gin")
            ps_b = ps1.tile([h, t_steps], F32, tag="psgb")
            nc.tensor.matmul(ps_b[:], wih[:, 2 * h:3 * h], xaug,
                             start=True, stop=True)
            nc.vector.tensor_copy(gi_n[:], ps_b[:])

            for t in range(t_steps):
                ps_rz = pspool.tile([2 * h, 1], F32, tag="psrz")
                nc.tensor.matmul(ps_rz[:], whh[:, 0:2 * h], haug[:],
                                 start=True, stop=False)
                nc.tensor.matmul(ps_rz[:], wih[:, 0:2 * h], xaug[:, t:t + 1],
                                 start=False, stop=True)
                ps_n = pspool.tile([h, 1], F32, tag="psn")
                nc.tensor.matmul(ps_n[:], whh[:, 2 * h:3 * h], haug[:],
                                 start=True, stop=True)
                sig = sm.tile([2 * h, 1], F32, tag="sig")
                nc.scalar.activation(sig[:], ps_rz[:], AF.Sigmoid)
                n_t = sm.tile([h, 1], F32, tag="nt")
                nc.scalar.activation(n_t[:], ps_n[:], AF.Tanh,
                                     bias=gi_n[:, t:t + 1], scale=sig[0:h, 0:1])
                hmn = sm.tile([h, 1], F32, tag="hmn")
                nc.scalar.activation(hmn[:], n_t[:], AF.Identity,
                                     bias=haug[0:h, 0:1], scale=-1.0)
                nc.scalar.activation(haug[0:h, :], hmn[:], AF.Identity,
                                     bias=n_t[:], scale=sig[h:2 * h, 0:1])

            ps_o = ps1.tile([8, 1], F32, tag="pso")
            nc.tensor.matmul(ps_o[:], hw, haug[:], start=True, stop=True)
            o = sm.tile([8, 1], F32, tag="o")
            nc.scalar.activation(o[:], ps_o[:], AF.Sigmoid)
            nc.sync.dma_start(out_d[:], o[:])
    nc.compile()
    return nc


_P1_CACHE = {}
_P2_CACHE = {}
TRACE = False
LAST_RES = {}
STAGE = 99


def _get_phase1(plan):
    key = plan.key() + (STAGE,)
    if key not in _P1_CACHE:
        nc = bacc.Bacc("TRN2", target_bir_lowering=False, debug=False,
                       num_devices=T)
        _P1_CACHE[key] = build_phase1(nc, plan, stage=STAGE)
    return _P1_CACHE[key]


def _get_phase2():
    if "p2" not in _P2_CACHE:
        nc = bacc.Bacc("TRN2", target_bir_lowering=False, debug=False,
                       num_devices=1)
        _P2_CACHE["p2"] = build_phase2(nc)
    return _P2_CACHE["p2"]


def kernel(x, edge_index, edge_weight, W1, b1, W2, b2, Wih, Whh, bih, bhh,
           headW, headb):
    x = np.asarray(x, np.float32)
    ei = np.asarray(edge_index)
    ew = np.asarray(edge_weight, np.float32)

    preps = [graph_prep(x[t], ei[t, 0].astype(np.int64),
                        ei[t, 1].astype(np.int64), ew[t]) for t in range(T)]
    plan = make_plan(preps)
    nc1 = _get_phase1(plan)
    in_maps = []
    for t in range(T):
        blob, blobx = pack_graph(plan, preps[t], W1, W2, b1, b2)
        in_maps.append({"pk": blob, "pkx": blobx})
    res1 = bass_utils.run_bass_kernel_spmd(nc1, in_maps,
                                           core_ids=list(range(T)),
                                           trace=TRACE)
    LAST_RES["p1"] = res1
    seq = np.stack([np.asarray(res1.results[t]["india"]).reshape(H)
                    for t in range(T)])  # [T, H]

    nc2 = _get_phase2()
    wih_a = np.concatenate([np.asarray(Wih, np.float32).T,
                            np.asarray(bih, np.float32)[None, :]], axis=0)
    whh_a = np.concatenate([np.asarray(Whh, np.float32).T,
                            np.asarray(bhh, np.float32)[None, :]], axis=0)
    hw_a = np.concatenate([np.asarray(headW, np.float32).T,
                           np.asarray(headb, np.float32)[None, :]], axis=0)
    xaug = np.concatenate([seq.T, np.ones((1, T), np.float32)], axis=0)
    h0 = np.zeros((H + 1, 1), np.float32)
    h0[H, 0] = 1.0
    fblk = np.concatenate([wih_a, whh_a, hw_a, xaug, h0], axis=1).astype(BF)
    blob2 = np.zeros((P, fblk.shape[1] * 2), np.uint8)
    blob2[0:H + 1] = np.ascontiguousarray(fblk).view(np.uint8)
    res2 = bass_utils.run_bass_kernel_spmd(nc2, [{"pk2": blob2}],
                                           core_ids=[0], trace=TRACE)
    LAST_RES["p2"] = res2
    return np.asarray(res2.results[0]["out"]).reshape(8).astype(np.float32)


# revision 5
# speedup vs baseline: 1.0334x; 1.0226x over previous
"""Trainium2 Bass kernel for nn_SanctionImpactGNN (2-hop pruned).

The model output is sigmoid(heads(GRU(seq))) where seq[t] = h2[node0] of a
2-layer GCN on graph t.  h2[node0] only depends on the 2-hop in-neighborhood
of node 0 (~350 edges of 320000), plus the weighted in-degrees (for the
symmetric normalization) of the ~350 involved nodes.

Host (indexing/packing only -- no FP math):
  * L1 = in-neighbors of node 0 (plus node 0 itself); layer-1 edge slots =
    all edges into L1 nodes + one self-loop slot per L1 node.
  * Per slot: the source node's x row (bf16 column of xT), in-edge weight
    lists of the slot's src and dst stacked along the partition (K) dim so a
    single matmul against a block-ones matrix computes all degree sums, the
    slot's own edge weight, and a one-hot dst indicator column.
  Everything is packed in ONE uint8 blob per graph -> one DMA per core.

Device phase 1 (SPMD, one graph per core):
  degs  = three tiny PE matmuls (lists^T @ block-ones) -> PSUM [128, 3nch+3]
  dis   = reciprocal(sqrt(1+degs))          (one ACT + one DVE op)
  norm  = dis[src]*dis[dst]*ew  per slot    (two DVE ops)
  msg_c = x[src]@W1 per slot chunk (bf16 matmul), copied to SBUF early
  psh1  = b1 + sum_c (A_c * norm_c)^T @ msg_c   (PSUM accumulation; b1 via
          ones-row x b1-row matmul)
  h1    = relu(psh1)                        (DVE tensor_relu)
  india = relu(b2 + dis0 * (W2^T @ (h1^T @ (disL1*s0))))  -> [64,1] f32

Device phase 2 (single core): GRU + heads (bf16 weights/h), per step:
  ps_rz = Whh_rz^T@h + Wih_rz^T@x_t   (PSUM accumulation, bias rows folded)
  sigmoid(ps_rz); tanh(ps_n*r + gi_n); h' = z*(h-n) + n
as 4 activation instructions + 3 tiny matmuls.
"""

import numpy as np
import ml_dtypes

import concourse.bacc as bacc
import concourse.mybir as mybir
import concourse.tile as tile
from concourse import bass_utils

F32 = mybir.dt.float32
BF16 = mybir.dt.bfloat16
U8 = mybir.dt.uint8
AF = mybir.ActivationFunctionType
OP = mybir.AluOpType
AX = mybir.AxisListType

T, N, E, F, H = 8, 20000, 320000, 128, 64
P = 128
INDIA = 0

BF = np.dtype(ml_dtypes.bfloat16)


def _align4(x):
    return (x + 3) & ~3


class Plan:
    """Compile-time sizes shared by all graphs (SPMD)."""

    def __init__(self, nch, nl1, k):
        self.nch, self.nl1, self.k = nch, nl1, k
        self.s1p = nch * P
        # degree-matmul grouping: gpm segments of length k per matmul (<=128
        # partitions each).  Segments: nch src + nch dst + degL1 + deg0 + s0.
        self.gpm = max(1, P // k)
        segs = 2 * nch + 3
        self.nmm = (segs + self.gpm - 1) // self.gpm
        self.segs = segs
        o = 0
        self.o_dl = o; o += self.nmm * P * 2             # bf16 [gpm*k, nmm*128]
        self.o_on = o; o += self.gpm * self.segs * 2     # bf16 ones blocks
        self.o_a = o; o = _align4(o + nch * nl1 * 2)     # bf16 [128, nch*NL1]
        self.o_ew = o; o += nch * 4                      # f32  [128, nch]
        self.o_w1 = o; o += H * 2                        # bf16 [128, 64]
        self.o_w2 = o; o += H * 2                        # bf16 [64, 64]
        self.o_b1 = o; o += H * 2                        # bf16 [1, 64] row
        self.o_or = o; o = _align4(o + 2)                # bf16 [1, 1] one
        self.o_b2 = o; o = _align4(o + 4)                # f32  [64, 1]
        self.pb = _align4(o)
        self.pbx = self.s1p * 2                          # bf16 [128, S1P] xT

    def key(self):
        return (self.nch, self.nl1, self.k)


def graph_prep(x_t, src, dst, ew):
    """Per-graph host extraction (pure indexing).  Returns raw structures."""
    order = np.argsort(dst, kind="stable")
    dst_s = dst[order]
    starts = np.searchsorted(dst_s, np.arange(N + 1))

    def in_edges(v):          # edge ids (original) into node v
        return order[starts[v]:starts[v + 1]]

    e0 = in_edges(INDIA)
    l1rest = np.unique(src[e0])
    l1rest = l1rest[l1rest != INDIA]
    L1 = np.concatenate([[INDIA], l1rest]).astype(np.int64)
    nl1 = len(L1)

    slot_src, slot_dst, slot_ew = [], [], []
    for j, v in enumerate(L1):
        eids = in_edges(v)
        slot_src.append(src[eids])
        slot_dst.append(np.full(len(eids) + 1, j, np.int64))
        slot_ew.append(ew[eids])
        # self-loop
        slot_src.append(np.array([v], np.int64))
        slot_ew.append(np.array([1.0], np.float32))
    slot_src = np.concatenate(slot_src).astype(np.int64)
    slot_dst = np.concatenate(slot_dst)
    slot_ew = np.concatenate(slot_ew).astype(np.float32)
    s1 = len(slot_src)

    involved = np.unique(np.concatenate([slot_src, L1]))
    indeg = starts[1:] - starts[:-1]
    kmax = int(indeg[involved].max()) if len(involved) else 1
    # s0 lists: edges L1[j] -> 0 (+ self-loop 1.0 for j==0)
    s0_lists = []
    src_e0 = src[e0]
    for j, v in enumerate(L1):
        vals = ew[e0[src_e0 == v]]
        if v == INDIA:
            vals = np.concatenate([vals, [np.float32(1.0)]])
        s0_lists.append(np.asarray(vals, np.float32))
    kmax = max(kmax, max(len(v) for v in s0_lists))
    return dict(L1=L1, nl1=nl1, slot_src=slot_src, slot_dst=slot_dst,
                slot_ew=slot_ew, s1=s1, kmax=kmax, s0_lists=s0_lists,
                in_edges=in_edges, ew=ew, x_t=x_t)


def make_plan(preps):
    s1 = max(p["s1"] for p in preps)
    nch = (s1 + P - 1) // P
    nl1 = max(p["nl1"] for p in preps)
    k = max(p["kmax"] for p in preps)
    return Plan(nch, nl1, k)


def pack_graph(plan, prep, W1, W2, b1, b2):
    nch, nl1, k = plan.nch, plan.nl1, plan.k
    gpm, segs, nmm = plan.gpm, plan.segs, plan.nmm
    s1p = plan.s1p
    blob = np.zeros((P, plan.pb), np.uint8)
    blobx = np.zeros((P, plan.pbx), np.uint8)

    def put(off, arr, rows=P):
        a = np.ascontiguousarray(arr)
        b = a.view(np.uint8).reshape(rows, -1)
        blob[:rows, off:off + b.shape[1]] = b

    ns = prep["s1"]
    ssrc, sdst, sew = prep["slot_src"], prep["slot_dst"], prep["slot_ew"]
    in_edges, ew = prep["in_edges"], prep["ew"]
    L1 = prep["L1"]

    # xT: [128 features, S1P slots] bf16 -> its own blob (second DMA)
    xt = np.zeros((P, s1p), BF)
    xt[:, :ns] = np.asarray(prep["x_t"], np.float32)[ssrc].T.astype(BF)
    blobx[:] = np.ascontiguousarray(xt).view(np.uint8)

    # degree list segments, each [k, 128] (cols = lanes):
    #   seg c       (c<nch):    in-ew list of src(slot c,p) at col p
    #   seg nch+c:              in-ew list of dst(slot c,p) at col p
    #   seg 2nch:               in-ew list of L1[p]  (cols < nl1)
    #   seg 2nch+1:             in-ew list of node 0 (all 64 cols)
    #   seg 2nch+2:             ew of edges L1[p] -> 0 (+1.0 at p==0)
    seglists = np.zeros((segs, k, P), BF)
    lane, chunk = np.arange(ns) % P, np.arange(ns) // P
    l1arr = L1[sdst]
    for s in range(ns):
        lst = ew[in_edges(ssrc[s])]
        seglists[chunk[s], :len(lst), lane[s]] = lst.astype(BF)
        lstd = ew[in_edges(l1arr[s])]
        seglists[nch + chunk[s], :len(lstd), lane[s]] = lstd.astype(BF)
    for j, v in enumerate(L1):
        lst = ew[in_edges(v)]
        seglists[2 * nch, :len(lst), j] = lst.astype(BF)
        s0l = prep["s0_lists"][j]
        seglists[2 * nch + 2, :len(s0l), j] = s0l.astype(BF)
    l0 = ew[in_edges(INDIA)].astype(BF)
    seglists[2 * nch + 1, :len(l0), :H] = l0[:, None]

    # stack gpm segments per matmul along partitions; ones blocks map each
    # K-rows group to its psum column.
    dlm = np.zeros((P, nmm * P), BF)
    onem = np.zeros((P, gpm * segs), BF)
    for si in range(segs):
        m, g = si // gpm, si % gpm
        dlm[g * k:(g + 1) * k, m * P:(m + 1) * P] = seglists[si]
        onem[g * k:(g + 1) * k, si] = np.float32(1.0)
    put(plan.o_dl, dlm)
    put(plan.o_on, onem)

    A = np.zeros((P, nch, nl1), BF)
    ewm = np.zeros((P, nch), np.float32)
    ewm[lane, chunk] = sew
    A[lane, chunk, sdst] = np.float32(1.0)
    put(plan.o_a, A.reshape(P, -1))
    put(plan.o_ew, ewm)

    put(plan.o_w1, np.asarray(W1, np.float32).astype(BF))            # [128,64]
    put(plan.o_w2, np.asarray(W2, np.float32).astype(BF), rows=H)    # [64,64]
    put(plan.o_b1, np.asarray(b1, np.float32).astype(BF).reshape(1, H), rows=1)
    put(plan.o_or, np.ones((1, 1), BF), rows=1)
    put(plan.o_b2, np.asarray(b2, np.float32).reshape(H, 1), rows=H)
    return blob, blobx


def build_phase1(nc, plan, stage=99):
    nch, nl1, k = plan.nch, plan.nl1, plan.k
    gpm, segs, nmm = plan.gpm, plan.segs, plan.nmm
    pk_d = nc.dram_tensor("pk", [P, plan.pb], U8, kind="ExternalInput")
    pkx_d = nc.dram_tensor("pkx", [P, plan.pbx], U8, kind="ExternalInput")
    india_d = nc.dram_tensor("india", [H, 1], F32, kind="ExternalOutput")
    dbg_d = None
    if stage < 99:
        dbg_d = nc.dram_tensor("dbg", [P, 256], F32, kind="ExternalOutput")

    with tile.TileContext(nc) as tc:
        with (
            tc.tile_pool(name="const", bufs=1) as const,
            tc.tile_pool(name="sm", bufs=1) as sm,
            tc.tile_pool(name="wm", bufs=3) as wm,
            tc.tile_pool(name="psm", bufs=3, space="PSUM") as psmp,
            tc.tile_pool(name="ps1", bufs=1, space="PSUM") as ps1,
        ):
            pk = const.tile([P, plan.pb], U8, tag="pk")
            nc.sync.dma_start(pk[:], pk_d[:])
            pkx = const.tile([P, plan.pbx], U8, tag="pkx")
            nc.sync.dma_start(pkx[:], pkx_d[:])

            xv = pkx[:, :].bitcast(BF16)
            dlv = pk[:, plan.o_dl:plan.o_dl + nmm * P * 2].bitcast(BF16)
            onv = pk[:, plan.o_on:plan.o_on + gpm * segs * 2].bitcast(BF16)
            av = pk[:, plan.o_a:plan.o_a + nch * nl1 * 2].bitcast(BF16)
            ewv = pk[:, plan.o_ew:plan.o_ew + nch * 4].bitcast(F32)
            w1v = pk[:, plan.o_w1:plan.o_w1 + H * 2].bitcast(BF16)
            w2v = pk[0:H, plan.o_w2:plan.o_w2 + H * 2].bitcast(BF16)
            b1v = pk[0:1, plan.o_b1:plan.o_b1 + H * 2].bitcast(BF16)
            onr = pk[0:1, plan.o_or:plan.o_or + 2].bitcast(BF16)
            b2v = pk[0:H, plan.o_b2:plan.o_b2 + 4].bitcast(F32)

            # --- degree sums via block-ones matmuls -> psd [128, segs]
            psd = ps1.tile([P, segs], F32, tag="psd")
            for m in range(nmm):
                lo, hi = m * gpm, min((m + 1) * gpm, segs)
                nsg = hi - lo
                nc.tensor.matmul(psd[:, lo:hi],
                                 dlv[0:gpm * k, m * P:(m + 1) * P],
                                 onv[0:gpm * k, lo:hi],
                                 start=True, stop=True)
            # dis = 1/sqrt(1+deg) for all but the raw-s0 column
            dsq = sm.tile([P, segs - 1], F32, tag="dsq")
            nc.scalar.activation(dsq[:], psd[:, 0:segs - 1], AF.Sqrt, bias=1.0)
            dis = sm.tile([P, segs - 1], F32, tag="dis")
            nc.vector.reciprocal(dis[:], dsq[:])
            # norm = dis_src * dis_dst * ew  [128, nch]
            norm = sm.tile([P, nch], F32, tag="norm")
            nc.vector.tensor_mul(norm[:], dis[:, 0:nch], dis[:, nch:2 * nch])
            nc.vector.tensor_mul(norm[:], norm[:], ewv)

            if stage == 0:
                z = sm.tile([P, 256], F32, tag="dbgt")
                nc.vector.memset(z[:], 0.0)
                nc.vector.tensor_copy(z[:, 0:segs], psd[:])
                nc.vector.tensor_copy(z[:, 16:16 + nch], norm[:])
                nc.vector.tensor_copy(z[0:H, 24:25], cvec[:])
                nc.sync.dma_start(dbg_d[:], z[:])

            # --- per-chunk messages; copies off critical path
            psh1 = ps1.tile([nl1, H], F32, tag="psh1")
            # b1 enters the accumulation first (no data deps beyond blob)
            nc.tensor.matmul(psh1[:], onr.broadcast_to((1, nl1)), b1v,
                             start=True, stop=False)
            # messages: matmuls now; PSUM->SBUF moves split so the DVE one
            # is emitted after the norm chain (keeps recip/norm/asc
            # uninterrupted on DVE), ACT takes the rest.
            psms, msgs = [], []
            for c in range(nch):
                psm = psmp.tile([P, H], F32, tag="psm")
                nc.tensor.matmul(psm[:], xv[:, c * P:(c + 1) * P], w1v,
                                 start=True, stop=True)
                psms.append(psm)
                msgs.append(wm.tile([P, H], BF16, tag="msg", name=f"msg{c}"))
            for c in range(nch):
                if c != 1:
                    nc.scalar.copy(msgs[c][:], psms[c][:])
            asc = sm.tile([P, nch * nl1], BF16, tag="asc")
            nc.vector.tensor_tensor(
                asc[:].rearrange("p (c v) -> p c v", v=nl1),
                av.rearrange("p (c v) -> p c v", v=nl1),
                norm[:].unsqueeze(2).broadcast_to((P, nch, nl1)),
                op=OP.mult)
            if nch > 1:
                nc.vector.tensor_copy(msgs[1][:], psms[1][:])
            for c in range(nch):
                nc.tensor.matmul(psh1[:], asc[:, c * nl1:(c + 1) * nl1],
                                 msgs[c][:], start=False, stop=(c == nch - 1))
            # cvec = disL1 * s0_raw (rows < nl1); on ACT so it doesn't
            # preempt the DVE norm->asc chain
            cvec = sm.tile([H, 1], BF16, tag="cvec")
            nc.scalar.activation(cvec[:], psd[0:H, segs - 1:segs], AF.Copy,
                                 scale=dis[0:H, 2 * nch:2 * nch + 1])

            h1 = sm.tile([nl1, H], BF16, tag="h1")
            nc.vector.tensor_relu(h1[:], psh1[:])

            if stage == 1:
                z = sm.tile([P, 256], F32, tag="dbgt")
                nc.vector.memset(z[:], 0.0)
                nc.vector.tensor_copy(z[0:nl1, 0:H], psh1[:])
                nc.sync.dma_start(dbg_d[:], z[:])

            ps_a = ps1.tile([H, 1], F32, tag="psa")
            nc.tensor.matmul(ps_a[:], h1[:], cvec[0:nl1, 0:1],
                             start=True, stop=True)
            agg1 = sm.tile([H, 1], BF16, tag="agg1")
            nc.vector.tensor_copy(agg1[:], ps_a[:])
            ps_h2 = ps1.tile([H, 1], F32, tag="psh2")
            nc.tensor.matmul(ps_h2[:], w2v, agg1[:], start=True, stop=True)
            india = sm.tile([H, 1], F32, tag="india")
            nc.scalar.activation(india[:], ps_h2[:], AF.Relu,
                                 scale=dis[0:H, 2 * nch + 1:2 * nch + 2],
                                 bias=b2v)
            nc.sync.dma_start(india_d[:], india[:])
    nc.compile()
    return nc


def build_phase2(nc):
    h, t_steps = H, T
    # packed bf16 blob rows 0..64: wihT_aug | whhT_aug | headWT_aug | xaug | h0
    w2cols = 3 * h + 3 * h + 8 + t_steps + 1
    pk_d = nc.dram_tensor("pk2", [P, w2cols * 2], U8, kind="ExternalInput")
    out_d = nc.dram_tensor("out", [8, 1], F32, kind="ExternalOutput")

    with tile.TileContext(nc) as tc:
        with (
            tc.tile_pool(name="const", bufs=1) as const,
            tc.tile_pool(name="sm", bufs=4) as sm,
            tc.tile_pool(name="ps", bufs=2, space="PSUM") as pspool,
            tc.tile_pool(name="ps1", bufs=1, space="PSUM") as ps1,
        ):
            pk = const.tile([P, w2cols * 2], U8, tag="pk2")
            nc.sync.dma_start(pk[:], pk_d[:])
            fv = pk[0:h + 1, :].bitcast(BF16)
            wih = fv[:, 0:3 * h]
            whh = fv[:, 3 * h:6 * h]
            hw = fv[:, 6 * h:6 * h + 8]
            xaug = fv[:, 6 * h + 8:6 * h + 8 + t_steps]

            # h lives in the blob: initial value [0]*64 + [1] is loaded with
            # the weights; the per-step update overwrites rows 0..63 in place.
            haug = fv[:, 6 * h + 8 + t_steps:6 * h + 9 + t_steps]
            gi_n = const.tile([h, t_steps], F32, tag="gin")
            ps_b = ps1.tile([h, t_steps], F32, tag="psgb")
            nc.tensor.matmul(ps_b[:], wih[:, 2 * h:3 * h], xaug,
                             start=True, stop=True)
            nc.vector.tensor_copy(gi_n[:], ps_b[:])

            for t in range(t_steps):
                ps_rz = pspool.tile([2 * h, 1], F32, tag="psrz")
                nc.tensor.matmul(ps_rz[:], whh[:, 0:2 * h], haug[:],
                                 start=True, stop=False)
                nc.tensor.matmul(ps_rz[:], wih[:, 0:2 * h], xaug[:, t:t + 1],
                                 start=False, stop=True)
                ps_n = pspool.tile([h, 1], F32, tag="psn")
                nc.tensor.matmul(ps_n[:], whh[:, 2 * h:3 * h], haug[:],
                                 start=True, stop=True)
                sig = sm.tile([2 * h, 1], F32, tag="sig")
                nc.scalar.activation(sig[:], ps_rz[:], AF.Sigmoid)
                n_t = sm.tile([h, 1], F32, tag="nt")
                nc.scalar.activation(n_t[:], ps_n[:], AF.Tanh,
                                     bias=gi_n[:, t:t + 1], scale=sig[0:h, 0:1])
                hmn = sm.tile([h, 1], F32, tag="hmn")
                nc.scalar.activation(hmn[:], n_t[:], AF.Identity,
                                     bias=haug[0:h, 0:1], scale=-1.0)
                nc.scalar.activation(haug[0:h, :], hmn[:], AF.Identity,
                                     bias=n_t[:], scale=sig[h:2 * h, 0:1])

            ps_o = ps1.tile([8, 1], F32, tag="pso")
            nc.tensor.matmul(ps_o[:], hw, haug[:], start=True, stop=True)
            o = sm.tile([8, 1], F32, tag="o")
            nc.scalar.activation(o[:], ps_o[:], AF.Sigmoid)
            nc.sync.dma_start(out_d[:], o[:])
    nc.compile()
    return nc


_P1_CACHE = {}
_P2_CACHE = {}
TRACE = False
LAST_RES = {}
STAGE = 99


def _get_phase1(plan):
    key = plan.key() + (STAGE,)
    if key not in _P1_CACHE:
        nc = bacc.Bacc("TRN2", target_bir_lowering=False, debug=False,
                       num_devices=T)
        _P1_CACHE[key] = build_phase1(nc, plan, stage=STAGE)
    return _P1_CACHE[key]


def _get_phase2():
    if "p2" not in _P2_CACHE:
        nc = bacc.Bacc("TRN2", target_bir_lowering=False, debug=False,
                       num_devices=1)
        _P2_CACHE["p2"] = build_phase2(nc)
    return _P2_CACHE["p2"]


def kernel(x, edge_index, edge_weight, W1, b1, W2, b2, Wih, Whh, bih, bhh,
           headW, headb):
    x = np.asarray(x, np.float32)
    ei = np.asarray(edge_index)
    ew = np.asarray(edge_weight, np.float32)

    preps = [graph_prep(x[t], ei[t, 0].astype(np.int64),
                        ei[t, 1].astype(np.int64), ew[t]) for t in range(T)]
    plan = make_plan(preps)
    nc1 = _get_phase1(plan)
    in_maps = []
    for t in range(T):
        blob, blobx = pack_graph(plan, preps[t], W1, W2, b1, b2)
        in_maps.append({"pk": blob, "pkx": blobx})
    res1 = bass_utils.run_bass_kernel_spmd(nc1, in_maps,
                                           core_ids=list(range(T)),
                                           trace=TRACE)
    LAST_RES["p1"] = res1
    seq = np.stack([np.asarray(res1.results[t]["india"]).reshape(H)
                    for t in range(T)])  # [T, H]

    nc2 = _get_phase2()
    wih_a = np.concatenate([np.asarray(Wih, np.float32).T,
                            np.asarray(bih, np.float32)[None, :]], axis=0)
    whh_a = np.concatenate([np.asarray(Whh, np.float32).T,
                            np.asarray(bhh, np.float32)[None, :]], axis=0)
    hw_a = np.concatenate([np.asarray(headW, np.float32).T,
                           np.asarray(headb, np.float32)[None, :]], axis=0)
    xaug = np.concatenate([seq.T, np.ones((1, T), np.float32)], axis=0)
    h0 = np.zeros((H + 1, 1), np.float32)
    h0[H, 0] = 1.0
    fblk = np.concatenate([wih_a, whh_a, hw_a, xaug, h0], axis=1).astype(BF)
    blob2 = np.zeros((P, fblk.shape[1] * 2), np.uint8)
    blob2[0:H + 1] = np.ascontiguousarray(fblk).view(np.uint8)
    res2 = bass_utils.run_bass_kernel_spmd(nc2, [{"pk2": blob2}],
                                           core_ids=[0], trace=TRACE)
    LAST_RES["p2"] = res2
    return np.asarray(res2.results[0]["out"]).reshape(8).astype(np.float32)
